# revision 18
# baseline (speedup 1.0000x reference)
"""Mutual channel attention (sparse_attention) TRN2 Bass kernel — v2 (bf16).

Problem: x1, x2 of shape (16, 512, 64, 64) fp32.
  q = x1.reshape(B, C, D), k = x2.reshape(B, C, D), D = 4096, scale = 1/64
  S   = q @ k^T * scale                       [B, 512, 512]
  outA = softmax_rows(S) @ k                  -> (16, 512, 64, 64)
  outB = softmax_rows(S^T) @ q                -> (16, 512, 64, 64)

Key algebra: without max-subtraction (scores ~ N(0,1), safe in fp32),
P = exp(S*scale) serves BOTH directions; only the normalization sums
differ (row sums of P for A, column sums of P for B).

v2 vs v1 (285 us baseline):
- All matmuls in bf16 (1 cyc/row, rel err ~3e-3 vs 2e-2 budget). The
  host casts inputs to bf16 and also provides q^T/k^T in a chunked
  layout, so the per-d-chunk PE transposes (40% extra PE work in v1)
  disappear entirely. Outputs return as bf16 and are upcast on host.
- PE work per batch is just the three 512x512x4096 GEMMs + 16 P-block
  transposes: ~199k cycles -> ~166 us/core for 2 batches at 2.4 GHz.
- DMA per core: 33.6 MB in + 16.8 MB out = 50.4 MB (~141 us at 358
  GB/s), fully overlapped with compute.

Sharding: pure data parallel, 2 batches per core across 8 cores.

Per-core per-batch schedule:
  1. qT/kT piece loads (sync ring), k/q natural loads (scalar ring).
  2. Scores: per 128-d-chunk, 4 accumulating matmuls into resident
     S psum banks (stationary = qT slice, moving = kT chunk).
  3. exp via ACT with fused *1/64 scale and fused row-sum (dir A).
  4. PE-transpose P -> P_ec with fused column-sum on the ACT copy-out
     (dir B); DVE reciprocals.
  5. out_a = P_ec.T @ k, cc-outer with [128,4096] bf16 staging per
     c-chunk, normalization folded into the PSUM->SBUF copy
     (alternating DVE/ACT), one 1 MB store per chunk. Then
     out_b = P_ce.T @ q symmetrically.
"""

import numpy as np

B, C, D = 16, 512, 4096
N_CORES = 8
B_PER_CORE = B // N_CORES  # 2
CC = C // 128  # 4 c-chunks
DC = D // 128  # 32 d-chunks
# qT/kT piece sizes in d-chunks: small first pieces so batch-0 scores can
# start early, then large pieces for DMA efficiency.
PIECES = [2, 2, 4, 8, 8, 8]
NP = len(PIECES)
PSTART = [sum(PIECES[:i]) for i in range(NP)]  # chunk offset of each piece
NG = D // 512  # 8 d-groups of 512 in the out phase

_COMPILED = {}


def _build():
    import concourse.mybir as mybir
    from concourse import bacc, tile

    f32 = mybir.dt.float32
    bf16 = mybir.dt.bfloat16
    AF = mybir.ActivationFunctionType
    ROWS = B_PER_CORE * C  # 1024

    nc = bacc.Bacc(None, target_bir_lowering=False)
    x1 = nc.declare_dram_parameter("x1", [ROWS, D], bf16, isOutput=False)
    x2 = nc.declare_dram_parameter("x2", [ROWS, D], bf16, isOutput=False)
    # Transposed copies as per-piece blocks: piece (b, a) occupies rows
    # [b*D + PSTART[a]*128, +128*PIECES[a]) p-major, i.e. row p*n + di of
    # the block holds x[b, :, (PSTART[a]+di)*128 + p]. Each SBUF
    # partition's piece data is then one contiguous 2-8 KB run on both
    # sides -> full-size DMA descriptors (1 KB descriptors measured 55
    # GB/s under queue contention vs ~360 GB/s for 8 KB ones).
    x1t = nc.declare_dram_parameter("x1t", [B_PER_CORE * D, C], bf16, isOutput=False)
    x2t = nc.declare_dram_parameter("x2t", [B_PER_CORE * D, C], bf16, isOutput=False)
    ident = nc.declare_dram_parameter("ident", [128, 128], bf16, isOutput=False)
    outA = nc.declare_dram_parameter("outA", [ROWS, D], bf16, isOutput=True)
    outB = nc.declare_dram_parameter("outB", [ROWS, D], bf16, isOutput=True)

    with tile.TileContext(nc) as tc:
        with (
            tc.tile_pool(name="const", bufs=1) as constp,
            tc.tile_pool(name="tp", bufs=1) as tp,
            tc.tile_pool(name="nat", bufs=1) as nat,
            tc.tile_pool(name="pp", bufs=1) as pp,
            tc.tile_pool(name="rp", bufs=2) as rp,
            tc.tile_pool(name="ost", bufs=3) as ost,
            tc.tile_pool(name="sps", bufs=1, space="PSUM") as sps,
            tc.tile_pool(name="ops", bufs=4, space="PSUM") as ops,
        ):
            idt = constp.tile([128, 128], bf16)
            nc.scalar.dma_start(idt[:], ident[:])
            # zeroed scratch feeding dummy warm-up matmuls (keeps the PE
            # HAM clock-gate at 8/8 across DMA waits and phase boundaries)
            warm = constp.tile([128, 512], bf16)
            nc.vector.memset(warm[:], 0)

            def pe_bridge(n, tag_hint):
                w_ps = ops.tile([128, 512], f32, tag="st", name=f"warm{tag_hint}")
                for i in range(n):
                    nc.tensor.matmul(
                        w_ps[:], warm[:, 0:128], warm[:], start=True, stop=True
                    )

            # ~2.6us of dummy PE work: bridges the NEFF preamble -> first
            # qT/kT piece arrival and warms the clock gate before scores.
            pe_bridge(12, "init")

            for b in range(B_PER_CORE):
                r0 = b * C
                # ---- all loads on the sync ring, in consumption order:
                # qT/kT pieces (scores) interleaved with k chunks (out_a)
                # at the spare-bandwidth ratio, then q chunks (out_b). One
                # ring = strict priority, no packet-RR bandwidth steal
                # from a second load queue.
                qT = [None] * NP
                kT = [None] * NP
                k = [None] * CC
                q = [None] * CC

                def n_rows(cc, _r0=r0):
                    return slice(_r0 + cc * 128, _r0 + (cc + 1) * 128)

                def load_piece(a, _b=b):
                    n = PIECES[a]
                    r = slice(
                        _b * D + PSTART[a] * 128,
                        _b * D + (PSTART[a] + n) * 128,
                    )
                    qt = tp.tile([128, n, C], bf16, tag=f"qT{a}", name=f"qT{a}")
                    nc.sync.dma_start(
                        qt[:], x1t[r, :].rearrange("(p w) c -> p w c", p=128)
                    )
                    kt = tp.tile([128, n, C], bf16, tag=f"kT{a}", name=f"kT{a}")
                    nc.sync.dma_start(
                        kt[:], x2t[r, :].rearrange("(p w) c -> p w c", p=128)
                    )
                    qT[a] = qt
                    kT[a] = kt

                def load_k(cc):
                    ktile = nat.tile([128, D], bf16, tag=f"k{cc}", name=f"k{cc}")
                    nc.sync.dma_start(ktile[:], x2[n_rows(cc), :])
                    k[cc] = ktile

                for a in (0, 1, 2):
                    load_piece(a)
                load_k(0)
                load_piece(3)
                load_k(1)
                load_piece(4)
                load_k(2)
                load_k(3)
                load_piece(5)
                for cc in range(CC):
                    qtile = nat.tile([128, D], bf16, tag=f"q{cc}", name=f"q{cc}")
                    nc.sync.dma_start(qtile[:], x1[n_rows(cc), :])
                    q[cc] = qtile

                # ---- scores: S_ce[cc] accumulates over 32 d-chunks ----
                s_ps = [
                    sps.tile([128, C], f32, tag=f"s{cc}", name=f"s{cc}")
                    for cc in range(CC)
                ]
                for dc in range(DC):
                    a = max(i for i in range(NP) if PSTART[i] <= dc)
                    di = dc - PSTART[a]
                    for cc in range(CC):
                        nc.tensor.matmul(
                            s_ps[cc][:],
                            qT[a][:, di, cc * 128 : (cc + 1) * 128],
                            kT[a][:, di, :],
                            start=(dc == 0),
                            stop=(dc == DC - 1),
                        )

                # ---- exp + row sums (direction A) ----
                p_ce = []
                rinv_a = []
                for cc in range(CC):
                    p = pp.tile([128, C], bf16, tag=f"pce{cc}", name=f"pce{cc}")
                    rs = rp.tile([128, 1], f32, tag=f"rsa{cc}", name=f"rsa{cc}")
                    nc.scalar.activation(
                        p[:], s_ps[cc][:], AF.Exp, scale=1.0 / 64.0, accum_out=rs[:]
                    )
                    ri = rp.tile([128, 1], f32, tag=f"ria{cc}", name=f"ria{cc}")
                    nc.vector.reciprocal(ri[:], rs[:])
                    p_ce.append(p)
                    rinv_a.append(ri)

                # bridge the ~2us exp latency so the PE clock stays warm
                pe_bridge(8, f"b{b}")

                # ---- transpose P -> P_ec + column sums (direction B) ----
                stg = [
                    ops.tile([128, C], bf16, tag="st", name=f"pt{ec}")
                    for ec in range(CC)
                ]
                for cc in range(CC):
                    for ec in range(CC):
                        nc.tensor.transpose(
                            stg[ec][:, cc * 128 : (cc + 1) * 128],
                            p_ce[cc][:, ec * 128 : (ec + 1) * 128],
                            idt[:],
                        )
                p_ec = []
                rinv_b = []
                for ec in range(CC):
                    p = pp.tile([128, C], bf16, tag=f"pec{ec}", name=f"pec{ec}")
                    rs = rp.tile([128, 1], f32, tag=f"rsb{ec}", name=f"rsb{ec}")
                    nc.scalar.activation(p[:], stg[ec][:], AF.Copy, accum_out=rs[:])
                    ri = rp.tile([128, 1], f32, tag=f"rib{ec}", name=f"rib{ec}")
                    nc.vector.reciprocal(ri[:], rs[:])
                    p_ec.append(p)
                    rinv_b.append(ri)

                # ---- out_a = (P_ec.T @ k) * rinv_a, cc-outer, wide staging ----
                for cc in range(CC):
                    csl = slice(cc * 128, (cc + 1) * 128)
                    oa = ost.tile([128, D], bf16, tag="ost", name=f"oa{cc}")
                    for g in range(NG):
                        dsl = slice(g * 512, (g + 1) * 512)
                        o_ps = ops.tile([128, 512], f32, tag="st", name="oa_ps")
                        for ec in range(CC):
                            nc.tensor.matmul(
                                o_ps[:],
                                p_ec[ec][:, csl],
                                k[ec][:, dsl],
                                start=(ec == 0),
                                stop=(ec == CC - 1),
                            )
                        if g % 2 == 0:
                            nc.vector.tensor_scalar_mul(
                                oa[:, dsl], o_ps[:], rinv_a[cc][:]
                            )
                        else:
                            nc.scalar.activation(
                                oa[:, dsl], o_ps[:], AF.Copy, scale=rinv_a[cc][:]
                            )
                        if g in (NG // 2 - 1, NG - 1):
                            hsl = slice(0, D // 2) if g < NG - 1 else slice(D // 2, D)
                            nc.scalar.dma_start(
                                outA[r0 + cc * 128 : r0 + (cc + 1) * 128, hsl],
                                oa[:, hsl],
                            )

                # ---- out_b = (P_ce.T @ q) * rinv_b, ec-outer, wide staging ----
                for ec in range(CC):
                    esl = slice(ec * 128, (ec + 1) * 128)
                    ob = ost.tile([128, D], bf16, tag="ost", name=f"ob{ec}")
                    for g in range(NG):
                        dsl = slice(g * 512, (g + 1) * 512)
                        o_ps = ops.tile([128, 512], f32, tag="st", name="ob_ps")
                        for cc in range(CC):
                            nc.tensor.matmul(
                                o_ps[:],
                                p_ce[cc][:, esl],
                                q[cc][:, dsl],
                                start=(cc == 0),
                                stop=(cc == CC - 1),
                            )
                        if g % 2 == 0:
                            nc.vector.tensor_scalar_mul(
                                ob[:, dsl], o_ps[:], rinv_b[ec][:]
                            )
                        else:
                            nc.scalar.activation(
                                ob[:, dsl], o_ps[:], AF.Copy, scale=rinv_b[ec][:]
                            )
                        if g in (NG // 2 - 1, NG - 1):
                            hsl = slice(0, D // 2) if g < NG - 1 else slice(D // 2, D)
                            nc.scalar.dma_start(
                                outB[r0 + ec * 128 : r0 + (ec + 1) * 128, hsl],
                                ob[:, hsl],
                            )

    nc.finalize()
    return nc


def _get_nc():
    if "nc" not in _COMPILED:
        _COMPILED["nc"] = _build()
    return _COMPILED["nc"]


def make_in_maps(x1: np.ndarray, x2: np.ndarray):
    import ml_dtypes

    bf = ml_dtypes.bfloat16
    x1 = np.asarray(x1, dtype=np.float32).reshape(B, C, D).astype(bf)
    x2 = np.asarray(x2, dtype=np.float32).reshape(B, C, D).astype(bf)

    # per-piece p-major transposed blocks; see kernel comment on x1t
    def transposed(x):
        xt = np.empty((B, D, C), dtype=bf)
        xc = x.reshape(B, C, DC, 128)
        for a, n in enumerate(PIECES):
            s = PSTART[a]
            xt[:, s * 128 : (s + n) * 128, :] = (
                xc[:, :, s : s + n, :].transpose(0, 3, 2, 1).reshape(B, n * 128, C)
            )
        return xt

    x1t = transposed(x1)
    x2t = transposed(x2)
    ident = np.eye(128, dtype=bf)

    in_maps = []
    for i in range(N_CORES):
        sl = slice(i * B_PER_CORE, (i + 1) * B_PER_CORE)
        in_maps.append(
            {
                "x1": np.ascontiguousarray(x1[sl].reshape(B_PER_CORE * C, D)),
                "x2": np.ascontiguousarray(x2[sl].reshape(B_PER_CORE * C, D)),
                "x1t": x1t[sl].reshape(B_PER_CORE * D, C),
                "x2t": x2t[sl].reshape(B_PER_CORE * D, C),
                "ident": ident,
            }
        )
    return in_maps


def kernel(x1: np.ndarray, x2: np.ndarray):
    from concourse.bass_utils import run_bass_kernel_spmd

    nc = _get_nc()
    in_maps = make_in_maps(x1, x2)

    res = None
    for attempt in range(3):
        try:
            res = run_bass_kernel_spmd(nc, in_maps, list(range(N_CORES))).results
            break
        except Exception:
            if attempt == 2:
                raise
    assert res is not None

    outA = np.empty((B, C, 64, 64), dtype=np.float32)
    outB = np.empty((B, C, 64, 64), dtype=np.float32)
    for i in range(N_CORES):
        sl = slice(i * B_PER_CORE, (i + 1) * B_PER_CORE)
        outA[sl] = res[i]["outA"].astype(np.float32).reshape(B_PER_CORE, C, 64, 64)
        outB[sl] = res[i]["outB"].astype(np.float32).reshape(B_PER_CORE, C, 64, 64)
    return outA, outB
